# revision 19
# baseline (speedup 1.0000x reference)
"""DFA-GNN (max-aggregation message passing) Trainium2 kernel.

Problem (B=2, N=4096, E=65536, M=4, H=256), per batch b:
    coeff[e]  = edge_fts[b,e,:] @ edge_W + edge_b                  # [E]
    agg[n]    = max over edges e with tgt[e]==n of coeff[e] * hint[b, src[e]]
    out[b,n]  = (node_fts[b,n] + agg[n]) @ update_W + update_b     # [M,H] rows

Sharding: 8 cores = 2 batches x 4 target-node quarters (1024 nodes each).
Edges are bucketed by target node on the host (every node has exactly 16
incoming edges with this generator; general counts <=16 are padded by
duplicating an edge, which preserves the max).

Data path is bf16 (harness gate is rel_err < 2e-2; measured ~4.7e-3):
  - gather: one SWDGE dma_gather per 4 edge ranks (512 descriptors per Pool
    instruction, single_packet, round-robin over 4 SWDGE queues) pulls 2KB
    hint rows from HBM at full DMA bandwidth into [128 nodes, 4*1024]
    tiles; 6 tiles in flight bound the pipeline lag (and so the drain
    tail) to ~1.5 blocks,
  - per-edge coefficients via PE matmuls (edge_W stationary, host-transposed
    edge features streaming), sprayed into a [128, 16] f32 per-partition
    layout (coeff for block nb+1 is emitted before block nb's chain),
  - mult+max chain on two accumulators: odd ranks multiply on the Act
    engine (out-of-place, per-partition f32 scale) and TT-max on DVE (2x
    mode); even ranks run the fused scalar_tensor_tensor on DVE (1x; the
    split load-balances DVE ~15.5us vs Act ~14.5us per block),
  - +node_fts as a DVE tensor_tensor add, PE transposes to feature-major,
    update_W matmuls with update_b folded in as a ones x b rank-1 term,
  - bf16 output, upcast to f32 on the host.

Engine budget per core (8 blocks): DMA ~134us (gather 94 + streams 40,
byte-bound at ~22GB/s/engine x16), DVE ~125us, Act ~115us, Pool ~40us of
real descriptor generation (the rest of its occupancy is ring
backpressure), PE ~96us. Measured ~176us end-to-end.
"""

import os
import sys

import numpy as np

for _p in ("/opt/trn_rl_repo", "/root/.axon_site/_ro/trn_rl_repo"):
    if os.path.isdir(_p) and _p not in sys.path:
        sys.path.insert(0, _p)

B, N, E, M, H = 2, 4096, 65536, 4, 256
MH = M * H            # 1024
P = 128               # partitions
K = 16                # edges per node (E // N)
NCORE = N // 4        # nodes per core (1024)
NB = NCORE // P       # node blocks per core (8)
EC = NCORE * K        # edges per core (16384)
ECB = P * K           # edges per block (2048)
KH = K // 4           # edge ranks per gather tile (4)
GT_BUFS = int(os.environ.get("KERNEL_GT_BUFS", "6"))
N_CORES = 8
N_SWDGE_Q = int(os.environ.get("KERNEL_SWDGE_Q", "4"))

_CACHE = {}

# Set by kernel() when KERNEL_TRACE=1: BassKernelResults of the last run.
LAST_RESULT = None


def _build():
    from concourse import bass, bacc, mybir, tile

    f32 = mybir.dt.float32
    i16 = mybir.dt.int16
    bf16 = mybir.dt.bfloat16

    nc = bacc.Bacc("TRN2", target_bir_lowering=False, debug=False,
                   num_devices=N_CORES, num_swdge_queues=N_SWDGE_Q)

    hint = nc.dram_tensor("hint", [N, MH], bf16, kind="ExternalInput")
    eftsT = nc.dram_tensor("eftsT", [H, EC], bf16, kind="ExternalInput")
    idx_d = nc.dram_tensor("idx16", [P, EC // 16], i16, kind="ExternalInput")
    nf_d = nc.dram_tensor("nf", [NCORE, MH], bf16, kind="ExternalInput")
    eW_d = nc.dram_tensor("eW", [P, 2], bf16, kind="ExternalInput")
    eb_d = nc.dram_tensor("eb", [P, 1], f32, kind="ExternalInput")
    uW_d = nc.dram_tensor("uW", [H, H], bf16, kind="ExternalInput")
    ub_d = nc.dram_tensor("ub", [1, H], bf16, kind="ExternalInput")
    out_d = nc.dram_tensor("out", [NCORE, MH], bf16, kind="ExternalOutput")

    with tile.TileContext(nc) as tc:
        from concourse.mybir import AluOpType as alu

        with (
            tc.tile_pool(name="const", bufs=1) as cpool,
            tc.tile_pool(name="efts", bufs=2) as epool,
            tc.tile_pool(name="gt", bufs=GT_BUFS) as gpool,
            tc.tile_pool(name="sc", bufs=3) as scpool,
            tc.tile_pool(name="work", bufs=2) as wpool,
            tc.tile_pool(name="ps_coeff", bufs=2, space="PSUM") as ps_coeff,
            tc.tile_pool(name="ps_xt", bufs=1, space="PSUM") as ps_xt,
            tc.tile_pool(name="ps_out", bufs=1, space="PSUM") as ps_out,
        ):
            from concourse.masks import make_identity

            idx_t = cpool.tile([P, EC // 16], i16)
            nc.sync.dma_start(out=idx_t[:], in_=idx_d[:])
            ident = cpool.tile([P, P], bf16)
            make_identity(nc, ident[:])
            eW = cpool.tile([P, 2], bf16)
            nc.sync.dma_start(out=eW[:], in_=eW_d[:])
            eb = cpool.tile([P, 1], f32)
            nc.sync.dma_start(out=eb[:], in_=eb_d[:])
            uW0 = cpool.tile([P, H], bf16)
            uW1 = cpool.tile([P, H], bf16)
            nc.sync.dma_start(out=uW0[:], in_=uW_d[0:P, :])
            nc.sync.dma_start(out=uW1[:], in_=uW_d[P:2 * P, :])
            ub_row = cpool.tile([1, H], bf16)
            nc.sync.dma_start(out=ub_row[:], in_=ub_d[:])
            ones1 = cpool.tile([1, P], bf16)
            nc.vector.memset(ones1[:], 1.0)

            coeffs = [None] * NB

            def emit_coeff(nb):
                # eftsT columns node-major in the block:
                # col nb*2048 + p*16 + k -> edge rank k of node p.
                efts0 = epool.tile([P, ECB], bf16, tag="efts0")
                efts1 = epool.tile([P, ECB], bf16, tag="efts1")
                nc.sync.dma_start(out=efts0[:],
                                  in_=eftsT[0:P, nb * ECB:(nb + 1) * ECB])
                nc.sync.dma_start(out=efts1[:],
                                  in_=eftsT[P:2 * P, nb * ECB:(nb + 1) * ECB])
                co_ps = ps_coeff.tile([128, 1024], f32, tag="co_ps",
                                      space="PSUM")
                for c in range(4):
                    pp, ff = (c % 2) * 64, (c // 2) * 512
                    nc.tensor.matmul(co_ps[pp:pp + 1, ff:ff + 512],
                                     lhsT=eW[:, 0:1],
                                     rhs=efts0[:, c * 512:(c + 1) * 512],
                                     start=True, stop=False)
                    nc.tensor.matmul(co_ps[pp:pp + 1, ff:ff + 512],
                                     lhsT=eW[:, 1:2],
                                     rhs=efts1[:, c * 512:(c + 1) * 512],
                                     start=False, stop=True)
                co_row = wpool.tile([P, 512], f32, tag="co_row")
                for c in range(4):
                    pp, ff = (c % 2) * 64, (c // 2) * 512
                    nc.scalar.add(co_row[c * 32:c * 32 + 1, :],
                                  co_ps[pp:pp + 1, ff:ff + 512], eb[0:1, 0:1])
                coeff = wpool.tile([P, K], f32, tag="coeff")
                for c in range(4):
                    nc.sync.dma_start(
                        out=coeff[c * 32:(c + 1) * 32, :],
                        in_=co_row[c * 32:c * 32 + 1, :].rearrange(
                            "c (p k) -> c p k", k=K))
                coeffs[nb] = coeff

            emit_coeff(0)
            for nb in range(NB):
                # ---- gather: one SWDGE dma_gather per 8 edge ranks ----
                gts = []
                for h in range(K // KH):
                    gt = gpool.tile([P, KH * MH], bf16, tag="gt")
                    c0 = (nb * ECB + h * P * KH) // 16
                    nc.gpsimd.dma_gather(
                        gt[:].rearrange("p (g e) -> p g e", e=MH),
                        hint[:],
                        idx_t[:, c0:c0 + P * KH // 16],
                        P * KH, P * KH, MH,
                        queue_num=(nb * (K // KH) + h) % N_SWDGE_Q,
                    )
                    gts.append(gt)
                nf = wpool.tile([P, MH], bf16, tag="nf")
                nc.scalar.dma_start(out=nf[:], in_=nf_d[nb * P:(nb + 1) * P, :])
                if nb + 1 < NB:
                    emit_coeff(nb + 1)

                # ---- mult+max chain, two accumulators (even/odd rank) ---
                # even ranks >=2 run fused STT on DVE; odd ranks multiply
                # out-of-place on Act, then TT-max on DVE (2x mode).
                coeff = coeffs[nb]
                acc_a = wpool.tile([P, MH], bf16, tag="acc_a")
                acc_b = wpool.tile([P, MH], bf16, tag="acc_b")
                accs = [acc_a, acc_b]
                for h in range(K // KH):
                    gt = gts[h]
                    for j in range(KH):
                        k = h * KH + j
                        src = gt[:, j * MH:(j + 1) * MH]
                        sc = coeff[:, k:k + 1]
                        acc = accs[k % 2]
                        if k == 0:
                            nc.vector.tensor_scalar(
                                out=acc[:], in0=src, scalar1=sc,
                                scalar2=None, op0=alu.mult)
                        elif k == 1:
                            nc.scalar.mul(acc[:], src, sc)
                        elif k % 2 == 1:
                            sct = scpool.tile([P, MH], bf16, tag="sct")
                            nc.scalar.mul(sct[:], src, sc)
                            nc.vector.tensor_tensor(out=acc[:], in0=sct[:],
                                                    in1=acc[:], op=alu.max)
                        else:
                            nc.vector.scalar_tensor_tensor(
                                out=acc[:], in0=src, scalar=sc,
                                in1=acc[:], op0=alu.mult, op1=alu.max)

                # ---- combine, +node_fts, transpose, update matmuls ----
                acc = acc_a
                nc.vector.tensor_tensor(out=acc[:], in0=acc_b[:], in1=acc[:],
                                        op=alu.max)
                nc.vector.tensor_tensor(out=acc[:], in0=acc[:], in1=nf[:],
                                        op=alu.add)
                xt_ps = ps_xt.tile([P, MH], bf16, tag="xt_ps", space="PSUM")
                for c in range(MH // P):
                    nc.tensor.matmul(xt_ps[:, c * P:(c + 1) * P],
                                     lhsT=acc[:, c * P:(c + 1) * P],
                                     rhs=ident[:], is_transpose=True,
                                     start=True, stop=True)
                xt = wpool.tile([P, MH], bf16, tag="xt")
                nc.scalar.copy(xt[:], xt_ps[:])

                o_ps = ps_out.tile([P, MH], f32, tag="o_ps", space="PSUM")
                for m in range(M):
                    nc.tensor.matmul(o_ps[:, m * H:(m + 1) * H],
                                     lhsT=xt[:, (2 * m) * P:(2 * m + 1) * P],
                                     rhs=uW0[:], start=True, stop=False)
                    nc.tensor.matmul(o_ps[:, m * H:(m + 1) * H],
                                     lhsT=xt[:, (2 * m + 1) * P:(2 * m + 2) * P],
                                     rhs=uW1[:], start=False, stop=False)
                    nc.tensor.matmul(o_ps[:, m * H:(m + 1) * H],
                                     lhsT=ones1[0:1, :], rhs=ub_row[0:1, :],
                                     start=False, stop=True)
                o = wpool.tile([P, MH], bf16, tag="o")
                nc.scalar.copy(o[:], o_ps[:])
                nc.scalar.dma_start(out=out_d[nb * P:(nb + 1) * P, :], in_=o[:])

    nc.compile()
    return nc


def _install_ntff_hook():
    """Register the axon NTFF profiling hook if this image's antenv lacks it.

    Mirrors what trn_boot does when ``antenv.axon_hooks`` exists. Safe no-op
    on failure — tracing is skipped, execution still works.
    """
    import types

    try:
        import antenv.axon_hooks  # noqa: F401
        return
    except ImportError:
        pass
    try:
        import antenv
        from trn_agent_boot.trn_boot import _ntff_profile_via_ctypes

        hook = _ntff_profile_via_ctypes("/opt/axon/libaxon_pjrt.so")
        mod = types.ModuleType("antenv.axon_hooks")
        state = {"hook": hook}
        mod.get_axon_ntff_profile_hook = lambda: state["hook"]
        mod.set_axon_ntff_profile_hook = lambda h: state.update(hook=h)
        sys.modules["antenv.axon_hooks"] = mod
        antenv.axon_hooks = mod
    except Exception as e:  # pragma: no cover - best effort
        print(f"ntff hook install failed: {e}", file=sys.stderr)


def _edge_grid(tgt_b):
    """[N, K] edge ids bucketed by target node, padded by duplication."""
    counts = np.bincount(tgt_b, minlength=N)
    if counts.max() > K or counts.min() < 1:
        raise ValueError(f"edge counts per node outside [1, {K}]: "
                         f"min={counts.min()} max={counts.max()}")
    order = np.argsort(tgt_b, kind="stable")
    if (counts == K).all():
        return order.reshape(N, K)
    pos = np.zeros(N + 1, np.int64)
    np.cumsum(counts, out=pos[1:])
    offs = np.minimum(np.arange(K)[None, :], (counts - 1)[:, None])
    return order[pos[:-1, None] + offs]


def kernel(**inputs):
    global LAST_RESULT
    import ml_dtypes
    from concourse.bass_utils import run_bass_kernel_spmd

    wdt = ml_dtypes.bfloat16

    cfg = np.asarray(inputs["cfg_indices_padded"])
    hint_state = np.asarray(inputs["hint_state"], dtype=np.float32)
    node_fts = np.asarray(inputs["node_fts"], dtype=np.float32)
    edge_fts = np.asarray(inputs["edge_fts"], dtype=np.float32)
    edge_W = np.asarray(inputs["edge_W"], dtype=np.float32)
    edge_b = np.asarray(inputs["edge_b"], dtype=np.float32)
    update_W = np.asarray(inputs["update_W"], dtype=np.float32)
    update_b = np.asarray(inputs["update_b"], dtype=np.float32)

    src = np.asarray(cfg[..., 0], dtype=np.int64)
    tgt = np.asarray(cfg[..., 1], dtype=np.int64)

    if "nc" not in _CACHE:
        _CACHE["nc"] = _build()
    nc = _CACHE["nc"]

    eW_in = np.ascontiguousarray(edge_W[:, 0].reshape(2, P).T).astype(wdt)
    eb_in = np.full((P, 1), edge_b[0], np.float32)
    ub_in = np.ascontiguousarray(update_b[None, :]).astype(wdt)
    uW_in = update_W.astype(wdt)

    in_maps = []
    for b in range(B):
        hint_b = np.ascontiguousarray(
            hint_state[b].reshape(N, MH)).astype(wdt)
        grid = _edge_grid(tgt[b])             # [N, K]
        srcg = src[b][grid]                   # [N, K]
        for q in range(4):
            g_q = grid[q * NCORE:(q + 1) * NCORE]    # [1024, K]
            s_q = srcg[q * NCORE:(q + 1) * NCORE]
            # gather index order: i = nb*2048 + k*128 + p, wrapped into
            # [16, EC/16] (idx16[r, c] = position c*16+r), tiled to 128 rows.
            gorder = s_q.reshape(NB, P, K).transpose(0, 2, 1)   # [nb, k, p]
            idx16 = np.ascontiguousarray(
                np.tile(gorder.reshape(EC // 16, 16).T, (8, 1))
            ).astype(np.int16)
            # edge-feature column order: j = nb*2048 + p*16 + k (node-major)
            eids = g_q.reshape(NB * P * K)
            efts_t = np.ascontiguousarray(edge_fts[b][eids].T).astype(wdt)
            nf_q = np.ascontiguousarray(
                node_fts[b, q * NCORE:(q + 1) * NCORE].reshape(NCORE, MH)
            ).astype(wdt)
            in_maps.append({
                "hint": hint_b,
                "eftsT": efts_t,
                "idx16": idx16,
                "nf": nf_q,
                "eW": eW_in,
                "eb": eb_in,
                "uW": uW_in,
                "ub": ub_in,
            })

    trace = bool(int(os.environ.get("KERNEL_TRACE", "0")))
    if trace:
        _install_ntff_hook()
    res = run_bass_kernel_spmd(nc, in_maps, core_ids=list(range(N_CORES)),
                               trace=trace)
    if trace:
        LAST_RESULT = res

    out = np.empty((B, N, M, H), np.float32)
    for b in range(B):
        for q in range(4):
            o = np.asarray(res.results[b * 4 + q]["out"], dtype=np.float32)
            out[b, q * NCORE:(q + 1) * NCORE] = o.reshape(NCORE, M, H)
    return out
